# revision 1
# baseline (speedup 1.0000x reference)
"""Spatial self-attention (SAGAN-style) kernel for 8 Trainium2 NeuronCores.

Math (per batch b):
    xf  = x[b].reshape(C, N)                       # C=256, N=4096
    qT  = wq @ xf + bq                             # [32, N]
    kT  = wk @ xf                                  # [32, N]  (bk dropped: a
                                                   #  per-query constant shift
                                                   #  of E cancels in softmax)
    V0  = g*wv @ xf                                # [C, N]   (g*bv folded into
                                                   #  the residual on host)
    E^T = kT.T @ qT                                # [m, n]  (keys on partitions)
    A'  = exp(E^T)          (no max-subtraction: |E| < 29, safe in fp32)
    s   = colsum(A')                               # softmax denominator
    out = g*(V0 @ A / s) + (x + g*bv)

Sharding: core i handles batch b = i//2, query half h = i%2 (2048 queries).
Each core computes kT / V^T for the full 4096 keys of its batch. The host
rotates xf per-core so the core's 2048 query columns always sit at columns
0..2047 (attention reductions are permutation-invariant over keys).

Device layout (O^T formulation, units of 4 key blocks x 512 queries):
  - wq/wk shipped 4x column-replicated [C,128] so the projection matmul
    emits qT/kT with the d-dim already replicated across all four 32-row
    groups (no SBUF replication DMAs).
  - E^T via 4-way row-packed K=32 matmuls (tile_position=(32r,0)) into a
    [128,2048] 4-bank PSUM tile -- each concurrent matmul owns one bank
    (concurrent PE writes into a single PSUM bank are fatal on HW).
  - ONE [128,2048] exp ACTIVATE per unit (amortizes the ~313-cycle ACT
    instruction overhead).
  - O^T = A'.T @ [gV^T | 1] accumulated in four [128,257] PSUM tiles per
    chunk-pair; the ones column yields the softmax denominator free.
  - software-pipelined emission E(u), exp(u), O(u-1): the PE streams the
    previous unit's O matmuls while ACT runs exp, so neither engine idles
    behind the single-buffered e tile. Steady state ~2.53us per unit =
    exp(1.97) + E foursome(0.39) + semaphore hops.
  - PSUM budget: tag "e" [128,2048] (4 banks) + tags "ot0".."ot3"
    [128,257] (4 banks) = all 8 banks; phase-1 projection PSUMs ride a
    5-slot ring over the same tags.
  - phase-1 extraction split between ACT (kT/half the vT copies; bias
    adds eliminated: bk cancels in softmax, g*bv is pre-added to the
    host-side residual x^T) and DVE (qT bias add, rest of vT).
  - dual HWDGE queues (SP + Activation) for input DMAs; residual and
    output in bf16 to halve HBM traffic; a GPSIMD-memset-fed PE warm-up
    burst beats the HAM throttle before phase 1.
  - walrus allows at most ONE semaphore wait per TPB instruction; Tile's
    surplus waits are legalized post-hoc (_strip_self_waits,
    _split_multi_waits).
"""

import ml_dtypes
import numpy as np

import concourse.bass as bass
import concourse.mybir as mybir
import concourse.tile as tile
from concourse.bass import ts
from concourse.bass_utils import run_bass_kernel_spmd

B, C, HH, WW = 4, 256, 64, 64
N = HH * WW          # 4096 spatial positions
D = 32               # C // 8 head dim
NCORES = 8
NQ = N * B // NCORES  # 2048 queries per core
MB = N // 128        # 32 key blocks
NCH = NQ // 256      # 8 query chunks of 256 per core
NG = N // 512        # 8 key groups of 4 blocks
NSL = NQ // 128      # 16 query slices of 128

F32 = mybir.dt.float32
BF16 = mybir.dt.bfloat16
AF = mybir.ActivationFunctionType
OP = mybir.AluOpType

VW = C + 2          # vT block width: 256 channels + ones col + pad


def _build():
    nc = bass.Bass()
    xfb = nc.declare_dram_parameter("xfb", [C, N], BF16, isOutput=False)
    xtq = nc.declare_dram_parameter("xtq", [NQ, C], BF16, isOutput=False)
    wq4 = nc.declare_dram_parameter("wq4", [C, 128], BF16, isOutput=False)
    wk4 = nc.declare_dram_parameter("wk4", [C, 128], BF16, isOutput=False)
    wvT = nc.declare_dram_parameter("wvT", [C, C], BF16, isOutput=False)
    bq4 = nc.declare_dram_parameter("bq4", [128, 1], F32, isOutput=False)
    outT = nc.declare_dram_parameter("outT", [NQ, C], BF16, isOutput=True)

    with tile.TileContext(nc) as tc:
        with (
            tc.tile_pool(name="const", bufs=1) as constp,
            tc.tile_pool(name="xfp", bufs=1) as xfp,
            tc.tile_pool(name="big", bufs=1) as bigp,
            tc.tile_pool(name="apool", bufs=6) as apool,
            tc.tile_pool(name="fin", bufs=2) as finp,
            tc.tile_pool(name="ps", bufs=1, space="PSUM") as psp,
        ):
            wq_t = [constp.tile([128, 128], BF16, name=f"wq{i}") for i in range(2)]
            wk_t = [constp.tile([128, 128], BF16, name=f"wk{i}") for i in range(2)]
            wv_t = [constp.tile([128, C], BF16, name=f"wv{i}") for i in range(2)]
            bq_t = constp.tile([128, 1], F32, name="bq")
            xf_t = [[xfp.tile([128, 512], BF16, name=f"xf{i}_{c}")
                     for c in range(NG)] for i in range(2)]
            kT_t = [bigp.tile([128, 512], BF16, name=f"kT{c}") for c in range(NG)]
            qT_t = [bigp.tile([128, 512], BF16, name=f"qT{c}") for c in range(4)]
            vT_t = [bigp.tile([128, 4 * VW], BF16, name=f"vT{g}") for g in range(NG)]
            xt = bigp.tile([128, NSL * C], BF16, name="xt")

            # --- input DMAs: weights first, xf chunks split across the two
            # HWDGE queues (SP + Activation), residual x^T last ---
            nc.sync.dma_start(wq_t[0][:], wq4[0:128, :])
            nc.sync.dma_start(wq_t[1][:], wq4[128:256, :])
            nc.sync.dma_start(wk_t[0][:], wk4[0:128, :])
            nc.sync.dma_start(wk_t[1][:], wk4[128:256, :])
            nc.sync.dma_start(bq_t[:], bq4[:, :])
            nc.sync.dma_start(wv_t[0][:], wvT[0:128, :])
            nc.sync.dma_start(wv_t[1][:], wvT[128:256, :])
            # PE warm-up weights: memset on GPSIMD so it clears during the
            # framework preamble and the warm-up matmuls start ~4.5us in
            wz = constp.tile([128, 512], BF16, name="wz")
            nc.gpsimd.memset(wz[:], 0.0)
            for c in range(NG):
                for i in range(2):
                    eng = nc.sync if i == 0 else nc.scalar
                    eng.dma_start(xf_t[i][c][:],
                                  xfb[i * 128:(i + 1) * 128, ts(c, 512)])
            # instant make-up DMAs after the xf stream: cover the cumulative
            # DMA-semaphore deficit left by non-incrementing static loads so
            # phase-1 consumers release at xf-completion instead of waiting
            # for the (late) residual transfer
            scr = constp.tile([1, 128], BF16, name="scr")
            for di in range(8):
                deng = nc.sync if di % 2 == 0 else nc.scalar
                deng.dma_start(scr[0:1, 16 * di:16 * (di + 1)],
                               xfb[0:1, ts(di, 16)])
            nc.scalar.dma_start(
                xt[:].rearrange("p (s c) -> p s c", c=C),
                xtq.rearrange("(s p) c -> p s c", p=128))
            for g in range(NG):
                nc.gpsimd.memset(vT_t[g][:, C::VW], 1.0)

            # PE warm-up: dense zero matmuls while input DMAs stream in, so
            # HAM unthrottles (K=8/8) before phase 1 issues real work
            pswu = psp.tile([128, 512], F32, tag="ot3", name="pswu")
            for w in range(12):
                nc.tensor.matmul(pswu[:], lhsT=wz[:, 0:128], rhs=wz[:],
                                 start=True, stop=True, skip_group_check=True)

            # phase-1 PSUM ring: 5 slots (ot0..ot3 = 1 bank each, e = 4 banks)
            _ring = ["ot0", "ot1", "ot2", "ot3", "e"]
            _rix = [0]

            def p1tile(shape, nm):
                t = psp.tile(shape, F32, tag=_ring[_rix[0] % 5], name=nm)
                _rix[0] += 1
                return t

            # --- phase 1: per key chunk c: DMA the chunk just-in-time (so
            # Tile's schedule-position DMA-sem thresholds stay minimal), then
            # qT/kT projections and V^T blocks. Extraction load balanced
            # across ACT and DVE. ---
            for c in range(NG):
                psk = p1tile([128, 512], f"psk{c}")
                for cb in range(2):
                    nc.tensor.matmul(
                        psk[:], lhsT=wk_t[cb][:], rhs=xf_t[cb][c][:],
                        start=(cb == 0), stop=(cb == 1), skip_group_check=True)
                nc.scalar.copy(kT_t[c][:], psk[:])
                if c < 4:
                    psq = p1tile([128, 512], f"psq{c}")
                    for cb in range(2):
                        nc.tensor.matmul(
                            psq[:], lhsT=wq_t[cb][:], rhs=xf_t[cb][c][:],
                            start=(cb == 0), stop=(cb == 1),
                            skip_group_check=True)
                    nc.vector.tensor_scalar_add(qT_t[c][:], psq[:],
                                                bq_t[:, 0:1])
                for rp in range(2):
                    p = 2 * c + rp
                    psv = p1tile([128, 512], f"psv{p}")
                    for rr in range(2):
                        r = 2 * rp + rr
                        for cb in range(2):
                            nc.tensor.matmul(
                                psv[:, ts(rr, C)],
                                lhsT=xf_t[cb][c][:, ts(r, 128)],
                                rhs=wv_t[cb][:],
                                start=(cb == 0), stop=(cb == 1),
                                skip_group_check=True)
                    dst = vT_t[c][:, 2 * rp * VW:(2 * rp + 2) * VW].rearrange(
                        "p (b w) -> p b w", w=VW)[:, :, 0:C]
                    srcap = psv[:].rearrange("p (b w) -> p b w", w=C)
                    if p % 8 < 3:
                        nc.scalar.copy(dst, srcap)
                    else:
                        nc.vector.tensor_copy(dst, srcap)


            # --- phase 2: units u = (cp, g) over 512-query chunk-pairs.
            # e spans 4 PSUM banks; the 4 row-packed E matmuls each own one
            # bank (concurrent single-port writes to one bank are illegal).
            # Software-pipelined by one unit: emit E(u), exp(u), then the
            # O matmuls of unit u-1 so the PE streams O(u-1) while ACT runs
            # exp(u) and never idles behind the single-buffered e tile. ---
            NU = (NCH // 2) * NG
            ot_cur = None
            pend = None     # (ot tiles, a tile, g, cp) of unit u-1

            def emit_o(p):
                pot, pa, pg, pcp = p
                for r in range(4):
                    m = 4 * pg + r
                    st, sp = (m == 0), (m == MB - 1)
                    for j in range(4):
                        nc.tensor.matmul(
                            pot[j][:],
                            lhsT=pa[:, 512 * r + 128 * j:
                                    512 * r + 128 * (j + 1)],
                            rhs=vT_t[pg][:, r * VW:r * VW + C + 1],
                            start=st, stop=sp, skip_group_check=True)
                if pg == NG - 1:
                    for j in range(4):
                        sl = 4 * pcp + j
                        rcp = finp.tile([128, 1], F32, tag="r", bufs=4,
                                        name=f"r{pcp}_{j}")
                        nc.vector.reciprocal(rcp[:], pot[j][:, C:C + 1])
                        t = finp.tile([128, C], F32, tag="t", bufs=4,
                                      name=f"t{pcp}_{j}")
                        last = (pcp == NCH // 2 - 1)
                        if last:
                            nc.scalar.activation(t[:], pot[j][:, 0:C],
                                                 AF.Copy, scale=rcp[:, 0:1])
                        else:
                            nc.vector.tensor_scalar_mul(t[:], pot[j][:, 0:C],
                                                        rcp[:, 0:1])
                        f = finp.tile([128, C], BF16, tag="f", bufs=4,
                                      name=f"f{pcp}_{j}")
                        nc.vector.tensor_tensor(f[:], t[:], xt[:, ts(sl, C)],
                                                OP.add)
                        oeng = nc.scalar if (last and j % 2) else nc.sync
                        oeng.dma_start(outT[sl * 128:(sl + 1) * 128, :],
                                       f[:])

            for u in range(NU):
                cp, g = u // NG, u % NG
                if g == 0:
                    ot_cur = [psp.tile([128, C + 1], F32, tag=f"ot{j}",
                                       name=f"ot{j}_{cp}") for j in range(4)]
                e = psp.tile([128, 2048], F32, tag="e", name=f"e{cp}_{g}")
                for r in range(4):
                    nc.tensor.matmul(
                        e[:, ts(r, 512)],
                        lhsT=kT_t[g][32 * r:32 * (r + 1), ts(r, 128)],
                        rhs=qT_t[cp][32 * r:32 * (r + 1), :],
                        start=True, stop=True, skip_group_check=True,
                        tile_position=(32 * r, 0),
                    )
                a = apool.tile([128, 2048], BF16, tag="a", name=f"a{cp}_{g}")
                nc.scalar.activation(a[:], e[:], AF.Exp)
                if pend is not None:
                    emit_o(pend)
                pend = (ot_cur, a, g, cp)
            emit_o(pend)
    _strip_self_waits(nc)
    _split_multi_waits(nc)
    return nc


_ENGINE_SEM_PREFIX = {
    "EngineType.PE": "PE_",
    "EngineType.DVE": "DVE_",
    "EngineType.Activation": "Activation_",
    "EngineType.Pool": "Pool_",
    "EngineType.SP": "SP_",
}


def _strip_self_waits(nc):
    """Drop same-engine semaphore waits from multi-wait TPB instructions.

    Walrus allows exactly one sync wait per TPB instruction. Tile emits
    redundant self-engine waits (WAW on pool-slot reuse, RAW from same-engine
    producers): each engine executes its queue in order, so a wait on the
    engine's own semaphore is always satisfied by program order. Dropping
    them collapses every instruction to at most one (cross-engine) wait.
    """
    for bb in nc.m.functions[0].blocks:
        for inst in bb.instructions:
            si = inst.sync_info
            if si is None:
                continue
            w = si.on_wait
            if len(w) <= 1 or inst.opcode == "Drain":
                continue
            pfx = _ENGINE_SEM_PREFIX.get(str(inst.engine))
            if pfx is None:
                continue
            kept = [x for x in w if not x.ant_name.startswith(pfx)]
            if kept and len(kept) < len(w):
                si.on_wait = kept


def _split_multi_waits(nc):
    """Walrus allows one sync wait per TPB instruction; move surplus waits
    onto dedicated single-wait Drain instructions inserted just before the
    offender (same engine, executes in order)."""
    import bass_rust
    cnt = 0
    for bb in nc.m.functions[0].blocks:
        il = bb.instructions
        i = 0
        while i < len(il):
            inst = il[i]
            si = inst.sync_info
            w = si.on_wait if si else []
            if len(w) > 1:
                for j, wait in enumerate(w[:-1]):
                    d = mybir.InstDrain(name=f"{inst.name}-w{j}", ins=[], outs=[],
                                        bass_is_fusable=False)
                    d.engine = inst.engine
                    d.sync_info = bass_rust.SyncInfo(on_wait=[wait], on_update=[])
                    il.insert(i, d)
                    i += 1
                    cnt += 1
                si.on_wait = [w[-1]]
            i += 1
    return cnt


_NC_CACHE = None


def _get_nc():
    global _NC_CACHE
    if _NC_CACHE is None:
        _NC_CACHE = _build()
    return _NC_CACHE


def kernel(x, wq, bq, wk, bk, wv, bv, gamma, _trace=False):
    f32 = lambda a: np.ascontiguousarray(np.asarray(a, dtype=np.float32))
    bf16 = lambda a: np.ascontiguousarray(np.asarray(a, dtype=np.float32)
                                          .astype(ml_dtypes.bfloat16))
    x = f32(x)
    g = float(np.asarray(gamma).reshape(-1)[0])
    xfull = x.reshape(B, C, N)
    shared = {
        "wq4": bf16(np.tile(np.asarray(wq).T, (1, 4))),
        "wk4": bf16(np.tile(np.asarray(wk).T, (1, 4))),
        "wvT": bf16((g * np.asarray(wv)).T),
        "bq4": f32(np.tile(np.asarray(bq).reshape(D, 1), (128 // D, 1))),
    }
    gbv_row = (g * np.asarray(bv, dtype=np.float32)).reshape(1, C)
    in_maps = []
    for core in range(NCORES):
        b, h = core // 2, core % 2
        m = dict(shared)
        if h == 0:
            xr = xfull[b]
        else:
            # rotate so this core's query half sits at columns 0..NQ-1;
            # key order is irrelevant (attention reduces over all keys)
            xr = np.concatenate([xfull[b][:, NQ:], xfull[b][:, :NQ]], axis=1)
        m["xfb"] = bf16(xr)
        m["xtq"] = bf16(xr[:, :NQ].T + gbv_row)
        in_maps.append(m)

    res = run_bass_kernel_spmd(_get_nc(), in_maps, list(range(NCORES)),
                               trace=_trace)
    full = np.empty((B, C, N), np.float32)
    for core in range(NCORES):
        b, h = core // 2, core % 2
        full[b][:, h * NQ:(h + 1) * NQ] = np.asarray(res.results[core]["outT"], dtype=np.float32).T
    out = full.reshape(B, C, HH, WW)
    if _trace:
        return out, res
    return out



# revision 3
# speedup vs baseline: 1.2129x; 1.2129x over previous
"""Spatial self-attention (SAGAN-style) kernel for 8 Trainium2 NeuronCores.

Math (per batch b):
    xf  = x[b].reshape(C, N)                       # C=256, N=4096
    qT  = wq @ xf + bq                             # [32, N]
    kT  = wk @ xf                                  # [32, N]  (bk dropped: a
                                                   #  per-query constant shift
                                                   #  of E cancels in softmax)
    V0  = g*wv @ xf                                # [C, N]   (g*bv folded into
                                                   #  the residual on host)
    E^T = kT.T @ qT                                # [m, n]  (keys on partitions)
    A'  = exp(E^T)          (no max-subtraction: |E| < 29, safe in fp32)
    s   = colsum(A')                               # softmax denominator
    out = g*(V0 @ A / s) + (x + g*bv)

Sharding: core i handles batch b = i//2, query half h = i%2 (2048 queries).
Each core computes kT / V^T for the full 4096 keys of its batch. The host
rotates xf per-core so the core's 2048 query columns always sit at columns
0..2047 (attention reductions are permutation-invariant over keys).

Device layout (O^T formulation, units of 4 key blocks x 512 queries):
  - wq/wk shipped 4x column-replicated [C,128] so the projection matmul
    emits qT/kT with the d-dim already replicated across all four 32-row
    groups (no SBUF replication DMAs).
  - E^T via 4-way row-packed K=32 matmuls (tile_position=(32r,0)) into a
    [128,2048] 4-bank PSUM tile -- each concurrent matmul owns one bank
    (concurrent PE writes into a single PSUM bank are fatal on HW).
  - ONE [128,2048] exp ACTIVATE per unit (amortizes the ~313-cycle ACT
    instruction overhead).
  - O^T = A'.T @ [gV^T | 1] accumulated in four [128,257] PSUM tiles per
    chunk-pair; the ones column yields the softmax denominator free.
  - software-pipelined emission E(u), exp(u), O(u-1): the PE streams the
    previous unit's O matmuls while ACT runs exp, so neither engine idles
    behind the single-buffered e tile. Steady state ~2.53us per unit =
    exp(1.97) + E foursome(0.39) + semaphore hops.
  - PSUM budget: tag "e" [128,2048] (4 banks) + tags "ot0".."ot3"
    [128,257] (4 banks) = all 8 banks; phase-1 projection PSUMs ride a
    5-slot ring over the same tags.
  - phase-1 extraction split between ACT (kT/half the vT copies; bias
    adds eliminated: bk cancels in softmax, g*bv is pre-added to the
    host-side residual x^T) and DVE (qT bias add, rest of vT).
  - dual HWDGE queues (SP + Activation) for input DMAs; residual and
    output in bf16 to halve HBM traffic; a GPSIMD-memset-fed PE warm-up
    burst beats the HAM throttle before phase 1.
  - walrus allows at most ONE semaphore wait per TPB instruction; Tile's
    surplus waits are legalized post-hoc (_strip_self_waits,
    _split_multi_waits).
"""

import ml_dtypes
import numpy as np

import concourse.bass as bass
import concourse.mybir as mybir
import concourse.tile as tile
from concourse.bass import ts
from concourse.bass_utils import run_bass_kernel_spmd

B, C, HH, WW = 4, 256, 64, 64
N = HH * WW          # 4096 spatial positions
D = 32               # C // 8 head dim
NCORES = 8
NQ = N * B // NCORES  # 2048 queries per core
MB = N // 128        # 32 key blocks
NCH = NQ // 256      # 8 query chunks of 256 per core
NG = N // 512        # 8 key groups of 4 blocks
NSL = NQ // 128      # 16 query slices of 128

F32 = mybir.dt.float32
BF16 = mybir.dt.bfloat16
AF = mybir.ActivationFunctionType
OP = mybir.AluOpType

VW = C + 2          # vT block width: 256 channels + ones col + pad


def _build():
    nc = bass.Bass()
    xfb = nc.declare_dram_parameter("xfb", [C, N], BF16, isOutput=False)
    xtq = nc.declare_dram_parameter("xtq", [NQ, C], BF16, isOutput=False)
    wq4 = nc.declare_dram_parameter("wq4", [C, 128], BF16, isOutput=False)
    wk4 = nc.declare_dram_parameter("wk4", [C, 128], BF16, isOutput=False)
    wvT = nc.declare_dram_parameter("wvT", [C, C], BF16, isOutput=False)
    bq4 = nc.declare_dram_parameter("bq4", [128, 1], F32, isOutput=False)
    outT = nc.declare_dram_parameter("outT", [NQ, C], BF16, isOutput=True)

    with tile.TileContext(nc) as tc:
        with (
            tc.tile_pool(name="const", bufs=1) as constp,
            tc.tile_pool(name="xfp", bufs=1) as xfp,
            tc.tile_pool(name="big", bufs=1) as bigp,
            tc.tile_pool(name="apool", bufs=6) as apool,
            tc.tile_pool(name="fin", bufs=2) as finp,
            tc.tile_pool(name="ps", bufs=1, space="PSUM") as psp,
        ):
            wq_t = [constp.tile([128, 128], BF16, name=f"wq{i}") for i in range(2)]
            wk_t = [constp.tile([128, 128], BF16, name=f"wk{i}") for i in range(2)]
            wv_t = [constp.tile([128, C], BF16, name=f"wv{i}") for i in range(2)]
            bq_t = constp.tile([128, 1], F32, name="bq")
            xf_t = [[xfp.tile([128, 512], BF16, name=f"xf{i}_{c}")
                     for c in range(NG)] for i in range(2)]
            kT_t = [bigp.tile([128, 512], BF16, name=f"kT{c}") for c in range(NG)]
            qT_t = [bigp.tile([128, 512], BF16, name=f"qT{c}") for c in range(4)]
            vT_t = [bigp.tile([128, 4 * VW], BF16, name=f"vT{g}") for g in range(NG)]
            xt = bigp.tile([128, NSL * C], BF16, name="xt")

            # --- input DMAs: weights first, xf chunks split across the two
            # HWDGE queues (SP + Activation), residual x^T last ---
            nc.sync.dma_start(wq_t[0][:], wq4[0:128, :])
            nc.sync.dma_start(wq_t[1][:], wq4[128:256, :])
            nc.sync.dma_start(wk_t[0][:], wk4[0:128, :])
            nc.sync.dma_start(wk_t[1][:], wk4[128:256, :])
            nc.sync.dma_start(bq_t[:], bq4[:, :])
            nc.sync.dma_start(wv_t[0][:], wvT[0:128, :])
            nc.sync.dma_start(wv_t[1][:], wvT[128:256, :])
            # PE warm-up weights: memset on GPSIMD so it clears during the
            # framework preamble and the warm-up matmuls start ~4.5us in
            wz = constp.tile([128, 512], BF16, name="wz")
            nc.gpsimd.memset(wz[:], 0.0)
            for c in range(NG):
                for i in range(2):
                    eng = nc.sync if i == 0 else nc.scalar
                    eng.dma_start(xf_t[i][c][:],
                                  xfb[i * 128:(i + 1) * 128, ts(c, 512)])
            # instant make-up DMAs after the xf stream: cover the cumulative
            # DMA-semaphore deficit left by non-incrementing static loads so
            # phase-1 consumers release at xf-completion instead of waiting
            # for the (late) residual transfer
            scr = constp.tile([1, 128], BF16, name="scr")
            for di in range(8):
                deng = nc.sync if di % 2 == 0 else nc.scalar
                deng.dma_start(scr[0:1, 16 * di:16 * (di + 1)],
                               xfb[0:1, ts(di, 16)])
            nc.scalar.dma_start(
                xt[:].rearrange("p (s c) -> p s c", c=C),
                xtq.rearrange("(s p) c -> p s c", p=128))
            for g in range(NG):
                nc.gpsimd.memset(vT_t[g][:, C::VW], 1.0)

            # PE warm-up: dense zero matmuls while input DMAs stream in, so
            # HAM unthrottles (K=8/8) before phase 1 issues real work
            pswu = psp.tile([128, 512], F32, tag="ot3", name="pswu")
            for w in range(12):
                nc.tensor.matmul(pswu[:], lhsT=wz[:, 0:128], rhs=wz[:],
                                 start=True, stop=True, skip_group_check=True)

            # phase-1 PSUM ring: 5 slots (ot0..ot3 = 1 bank each, e = 4 banks)
            _ring = ["ot0", "ot1", "ot2", "ot3", "e"]
            _rix = [0]

            def p1tile(shape, nm):
                t = psp.tile(shape, F32, tag=_ring[_rix[0] % 5], name=nm)
                _rix[0] += 1
                return t

            # --- phase 1: per key chunk c: DMA the chunk just-in-time (so
            # Tile's schedule-position DMA-sem thresholds stay minimal), then
            # qT/kT projections and V^T blocks. Extraction load balanced
            # across ACT and DVE. ---
            for c in range(NG):
                psk = p1tile([128, 512], f"psk{c}")
                for cb in range(2):
                    nc.tensor.matmul(
                        psk[:], lhsT=wk_t[cb][:], rhs=xf_t[cb][c][:],
                        start=(cb == 0), stop=(cb == 1), skip_group_check=True)
                nc.scalar.copy(kT_t[c][:], psk[:])
                if c < 4:
                    psq = p1tile([128, 512], f"psq{c}")
                    for cb in range(2):
                        nc.tensor.matmul(
                            psq[:], lhsT=wq_t[cb][:], rhs=xf_t[cb][c][:],
                            start=(cb == 0), stop=(cb == 1),
                            skip_group_check=True)
                    nc.vector.tensor_scalar_add(qT_t[c][:], psq[:],
                                                bq_t[:, 0:1])
                for rp in range(2):
                    p = 2 * c + rp
                    psv = p1tile([128, 512], f"psv{p}")
                    for rr in range(2):
                        r = 2 * rp + rr
                        for cb in range(2):
                            nc.tensor.matmul(
                                psv[:, ts(rr, C)],
                                lhsT=xf_t[cb][c][:, ts(r, 128)],
                                rhs=wv_t[cb][:],
                                start=(cb == 0), stop=(cb == 1),
                                skip_group_check=True)
                    dst = vT_t[c][:, 2 * rp * VW:(2 * rp + 2) * VW].rearrange(
                        "p (b w) -> p b w", w=VW)[:, :, 0:C]
                    srcap = psv[:].rearrange("p (b w) -> p b w", w=C)
                    if p % 8 < 3:
                        nc.scalar.copy(dst, srcap)
                    else:
                        nc.vector.tensor_copy(dst, srcap)


            # --- phase 2: units u = (cp, g) over 512-query chunk-pairs.
            # e spans 4 PSUM banks; the 4 row-packed E matmuls each own one
            # bank (concurrent single-port writes to one bank are illegal).
            # Software-pipelined by one unit: emit E(u), exp(u), then the
            # O matmuls of unit u-1 so the PE streams O(u-1) while ACT runs
            # exp(u) and never idles behind the single-buffered e tile. ---
            NU = (NCH // 2) * NG
            ot_cur = None
            pend = None     # (ot tiles, a tile, g, cp) of unit u-1

            def emit_o(p):
                pot, pa, pg, pcp = p
                for r in range(4):
                    m = 4 * pg + r
                    st, sp = (m == 0), (m == MB - 1)
                    for j in range(4):
                        nc.tensor.matmul(
                            pot[j][:],
                            lhsT=pa[:, 512 * r + 128 * j:
                                    512 * r + 128 * (j + 1)],
                            rhs=vT_t[pg][:, r * VW:r * VW + C + 1],
                            start=st, stop=sp, skip_group_check=True)
                if pg == NG - 1:
                    for j in range(4):
                        sl = 4 * pcp + j
                        rcp = finp.tile([128, 1], F32, tag="r", bufs=4,
                                        name=f"r{pcp}_{j}")
                        nc.vector.reciprocal(rcp[:], pot[j][:, C:C + 1])
                        t = finp.tile([128, C], F32, tag="t", bufs=4,
                                      name=f"t{pcp}_{j}")
                        last = (pcp == NCH // 2 - 1)
                        if last:
                            nc.scalar.activation(t[:], pot[j][:, 0:C],
                                                 AF.Copy, scale=rcp[:, 0:1])
                        else:
                            nc.vector.tensor_scalar_mul(t[:], pot[j][:, 0:C],
                                                        rcp[:, 0:1])
                        f = finp.tile([128, C], BF16, tag="f", bufs=4,
                                      name=f"f{pcp}_{j}")
                        nc.vector.tensor_tensor(f[:], t[:], xt[:, ts(sl, C)],
                                                OP.add)
                        oeng = nc.scalar if (last and j % 2) else nc.sync
                        oeng.dma_start(outT[sl * 128:(sl + 1) * 128, :],
                                       f[:])

            for u in range(NU):
                cp, g = u // NG, u % NG
                if g == 0:
                    ot_cur = [psp.tile([128, C + 1], F32, tag=f"ot{j}",
                                       name=f"ot{j}_{cp}") for j in range(4)]
                e = psp.tile([128, 2048], F32, tag="e", name=f"e{cp}_{g}")
                for r in range(4):
                    nc.tensor.matmul(
                        e[:, ts(r, 512)],
                        lhsT=kT_t[g][32 * r:32 * (r + 1), ts(r, 128)],
                        rhs=qT_t[cp][32 * r:32 * (r + 1), :],
                        start=True, stop=True, skip_group_check=True,
                        tile_position=(32 * r, 0),
                    )
                a = apool.tile([128, 2048], BF16, tag="a", name=f"a{cp}_{g}")
                nc.scalar.activation(a[:], e[:], AF.Exp)
                if pend is not None:
                    emit_o(pend)
                pend = (ot_cur, a, g, cp)
            emit_o(pend)
    _strip_self_waits(nc)
    _strip_redundant_mm_incs(nc)
    _split_multi_waits(nc)
    return nc


_ENGINE_SEM_PREFIX = {
    "EngineType.PE": "PE_",
    "EngineType.DVE": "DVE_",
    "EngineType.Activation": "Activation_",
    "EngineType.Pool": "Pool_",
    "EngineType.SP": "SP_",
}


def _strip_self_waits(nc):
    """Drop same-engine semaphore waits from multi-wait TPB instructions.

    Walrus allows exactly one sync wait per TPB instruction. Tile emits
    redundant self-engine waits (WAW on pool-slot reuse, RAW from same-engine
    producers): each engine executes its queue in order, so a wait on the
    engine's own semaphore is always satisfied by program order. Dropping
    them collapses every instruction to at most one (cross-engine) wait.
    """
    for bb in nc.m.functions[0].blocks:
        for inst in bb.instructions:
            si = inst.sync_info
            if si is None:
                continue
            w = si.on_wait
            if len(w) <= 1 or inst.opcode == "Drain":
                continue
            pfx = _ENGINE_SEM_PREFIX.get(str(inst.engine))
            if pfx is None:
                continue
            kept = [x for x in w if not x.ant_name.startswith(pfx)]
            if kept and len(kept) < len(w):
                si.on_wait = kept


def _split_multi_waits(nc):
    """Walrus allows one sync wait per TPB instruction; move surplus waits
    onto dedicated single-wait Drain instructions inserted just before the
    offender (same engine, executes in order)."""
    import bass_rust
    cnt = 0
    for bb in nc.m.functions[0].blocks:
        il = bb.instructions
        i = 0
        while i < len(il):
            inst = il[i]
            si = inst.sync_info
            w = si.on_wait if si else []
            if len(w) > 1:
                for j, wait in enumerate(w[:-1]):
                    d = mybir.InstDrain(name=f"{inst.name}-w{j}", ins=[], outs=[],
                                        bass_is_fusable=False)
                    d.engine = inst.engine
                    d.sync_info = bass_rust.SyncInfo(on_wait=[wait], on_update=[])
                    il.insert(i, d)
                    i += 1
                    cnt += 1
                si.on_wait = [w[-1]]
            i += 1
    return cnt


def _strip_redundant_mm_incs(nc):
    """Drop per-matmul semaphore increments that no wait references.

    Tile emits `then_inc(PE_sem, 1)` on every matmul; each inc serializes
    ~26ns on the PE (EVT_SEM register write). Matmuls complete in pc order,
    so an inc is only needed at cumulative positions some wait references.
    Keep those, strip the rest, and remap every wait threshold to the new
    cumulative numbering.
    """
    from collections import defaultdict

    # 1. collect referenced thresholds per semaphore (across all engines)
    refd = defaultdict(set)
    for bb in nc.m.functions[0].blocks:
        for inst in bb.instructions:
            si = inst.sync_info
            if si is None:
                continue
            for w in si.on_wait:
                if w.wait_value is not None:
                    refd[w.ant_name].add(w.wait_value)

    # 2. walk PE matmuls in program order; strip unreferenced incs
    sem_count = defaultdict(int)
    kept_count = defaultdict(int)
    remap = {}  # sem -> {old_threshold: new_threshold}
    for bb in nc.m.functions[0].blocks:
        for inst in bb.instructions:
            if inst.opcode != "Matmult":
                continue
            si = inst.sync_info
            if si is None or not si.on_update:
                continue
            keep = []
            for u in si.on_update:
                s = u.ant_name
                if not s.startswith("PE_") or u.update_value != 1:
                    keep.append(u)
                    continue
                sem_count[s] += 1
                i = sem_count[s]
                if i in refd[s]:
                    kept_count[s] += 1
                    remap.setdefault(s, {})[i] = kept_count[s]
                    keep.append(u)
            si.on_update = keep

    # 3. remap wait thresholds on stripped semaphores
    for bb in nc.m.functions[0].blocks:
        for inst in bb.instructions:
            si = inst.sync_info
            if si is None:
                continue
            for w in si.on_wait:
                s = w.ant_name
                if s in remap and w.wait_value in remap[s]:
                    w.wait_value = remap[s][w.wait_value]


_NC_CACHE = None


def _get_nc():
    global _NC_CACHE
    if _NC_CACHE is None:
        _NC_CACHE = _build()
    return _NC_CACHE


def kernel(x, wq, bq, wk, bk, wv, bv, gamma, _trace=False):
    f32 = lambda a: np.ascontiguousarray(np.asarray(a, dtype=np.float32))
    bf16 = lambda a: np.ascontiguousarray(np.asarray(a, dtype=np.float32)
                                          .astype(ml_dtypes.bfloat16))
    x = f32(x)
    g = float(np.asarray(gamma).reshape(-1)[0])
    xfull = x.reshape(B, C, N)
    shared = {
        "wq4": bf16(np.tile(np.asarray(wq).T, (1, 4))),
        "wk4": bf16(np.tile(np.asarray(wk).T, (1, 4))),
        "wvT": bf16((g * np.asarray(wv)).T),
        "bq4": f32(np.tile(np.asarray(bq).reshape(D, 1), (128 // D, 1))),
    }
    gbv_row = (g * np.asarray(bv, dtype=np.float32)).reshape(1, C)
    in_maps = []
    for core in range(NCORES):
        b, h = core // 2, core % 2
        m = dict(shared)
        if h == 0:
            xr = xfull[b]
        else:
            # rotate so this core's query half sits at columns 0..NQ-1;
            # key order is irrelevant (attention reduces over all keys)
            xr = np.concatenate([xfull[b][:, NQ:], xfull[b][:, :NQ]], axis=1)
        m["xfb"] = bf16(xr)
        m["xtq"] = bf16(xr[:, :NQ].T + gbv_row)
        in_maps.append(m)

    res = run_bass_kernel_spmd(_get_nc(), in_maps, list(range(NCORES)),
                               trace=_trace)
    full = np.empty((B, C, N), np.float32)
    for core in range(NCORES):
        b, h = core // 2, core % 2
        full[b][:, h * NQ:(h + 1) * NQ] = np.asarray(res.results[core]["outT"], dtype=np.float32).T
    out = full.reshape(B, C, HH, WW)
    if _trace:
        return out, res
    return out



# revision 7
# speedup vs baseline: 1.2661x; 1.0438x over previous
"""Spatial self-attention (SAGAN-style) kernel for 8 Trainium2 NeuronCores.

Math (per batch b):
    xf  = x[b].reshape(C, N)                       # C=256, N=4096
    qT  = wq @ xf + bq                             # [32, N]
    kT  = wk @ xf                                  # [32, N]  (bk dropped: per-query
                                                   #  const shift cancels in softmax)
    V0  = g*wv @ xf                                # [C, N]   (g*bv folded into
                                                   #  the residual on host)
    E^T = kT.T @ qT                                # [keys, queries]
    A'  = exp(E^T)          (no max-subtraction: |E| < 29, safe in fp32)
    s   = colsum(A')                               # softmax denominator
    out = (V0 @ A / s) + (x + g*bv)

Sharding: core i handles batch b = i//2, query half h = i%2 (2048 queries).
The host rotates xf per-core so the core's 2048 query columns sit at
columns 0..2047 (attention reductions are permutation-invariant over keys).

Device pipeline (v2):
  - Inputs ship as 4 large DMAs with 2-8KB/partition rows (per-DMA fixed
    cost ~1.4us dominates small transfers): a [128,1026] weight blob
    (wq|wk|wv halves + bq column, all bf16), xf as two [128,4096] halves
    on the two HWDGE queues, and the pre-swizzled residual x^T+g*bv as
    one [128,4096] SWDGE transfer on the gpsimd queue.
  - Phase 1: per 512-key chunk: qT/kT projections (wq/wk shipped 4x
    column-replicated so the d-dim is pre-broadcast across the four
    32-row groups) and V^T blocks with a free ones column (VW stride).
    Extraction split between ACT and DVE.
  - Phase 2: 64 units u=(cp, g): cp = 512-query chunk, g = 2-key-block
    group. Units run in PAIRS (g even/odd) with SEPARATE PSUM e-tiles
    ("ea"/"ed", [128,1024] = 2 banks each): one 4-way row-packed E
    foursome fills both units' e-tiles (4 distinct PSUM banks), then
    the EVEN unit's exp runs on ACT (table exp) while the ODD unit's
    exp runs CONCURRENTLY on DVE as a Schraudolph bit-trick:
        bf16_bits(exp(x)) ~= int16(x * 128*log2(e) + 127*128 - c)
    (one tensor_scalar mult+add, round-to-nearest f32->int16, written
    through a .bitcast(int16) view of the bf16 a-tile; max rel err
    ~3.3% at c=5.5 -- washes out to <1e-3 in the final output).
    With the two exp engines alternating, the PE never waits on a
    single-buffered e-tile and runs O matmuls back-to-back.
  - O^T accumulated in four [128,257] PSUM tiles per cp; the ones
    column yields the softmax denominator for free. Emission per cp
    writes scaled+residual-added slices into a [128,1024] staging tile
    shipped as ONE output DMA per cp (host unswizzles).
  - PSUM budget: ea(2) + ed(2) + ot0..ot3(4) = 8 banks; phase-1
    projection PSUMs ride a ring over the same tags.
  - Post-processing: walrus allows one semaphore wait per TPB
    instruction (_strip_self_waits, _split_multi_waits), and Tile's
    per-matmul then_inc costs ~26ns each on the PE -- all increments
    at cumulative positions no wait references are stripped and the
    remaining thresholds remapped (_strip_redundant_mm_incs).
"""

import ml_dtypes
import numpy as np

import concourse.bass as bass
import concourse.mybir as mybir
import concourse.tile as tile
from concourse.bass import ts
from concourse.bass_utils import run_bass_kernel_spmd

B, C, HH, WW = 4, 256, 64, 64
N = HH * WW          # 4096 spatial positions
D = 32               # C // 8 head dim
NCORES = 8
NQ = N * B // NCORES  # 2048 queries per core
MB = N // 128        # 32 key blocks
NCP = NQ // 512      # 4 query chunks of 512 per core
NG = N // 512        # 8 vT key groups of 4 blocks
NU = N // 256        # 16 units of 2 key blocks per chunk

F32 = mybir.dt.float32
BF16 = mybir.dt.bfloat16
I16 = mybir.dt.int16
AF = mybir.ActivationFunctionType
OP = mybir.AluOpType

VW = C + 2          # vT block width: 256 channels + ones col + pad
WBLOB = 4 * 128 + 2 * 256 + 2   # wq0|wq1|wk0|wk1|wv0|wv1|bq|pad

# Schraudolph constants: int16(E * 128*log2e + (127*128 - c)) viewed as bf16
SCH_SCALE = 1.4426950408889634 * 128.0
SCH_BIAS = 127.0 * 128.0 - 5.5


def _build():
    nc = bass.Bass()
    wts = nc.declare_dram_parameter("wts", [128, WBLOB], BF16, isOutput=False)
    xf0 = nc.declare_dram_parameter("xf0", [128, N], BF16, isOutput=False)
    xf1 = nc.declare_dram_parameter("xf1", [128, N], BF16, isOutput=False)
    xts = nc.declare_dram_parameter("xts", [128, 16 * C], BF16, isOutput=False)
    outS = nc.declare_dram_parameter("outS", [128, NCP * 1024], BF16,
                                     isOutput=True)

    with tile.TileContext(nc) as tc:
        with (
            tc.tile_pool(name="const", bufs=1) as constp,
            tc.tile_pool(name="big", bufs=1) as bigp,
            tc.tile_pool(name="apool", bufs=6) as apool,
            tc.tile_pool(name="fin", bufs=2) as finp,
            tc.tile_pool(name="ps", bufs=1, space="PSUM") as psp,
        ):
            wb = constp.tile([128, WBLOB], BF16, name="wb")
            xf_t = [constp.tile([128, N], BF16, name=f"xf{i}") for i in range(2)]
            kT_t = [bigp.tile([128, 512], BF16, name=f"kT{c}") for c in range(NG)]
            qT_t = [bigp.tile([128, 512], BF16, name=f"qT{c}") for c in range(NCP)]
            vT_t = [bigp.tile([128, 4 * VW], BF16, name=f"vT{g}") for g in range(NG)]
            xt = bigp.tile([128, 16 * C], BF16, name="xt")

            wq_v = [wb[:, 128 * i:128 * (i + 1)] for i in range(2)]
            wk_v = [wb[:, 256 + 128 * i:256 + 128 * (i + 1)] for i in range(2)]
            wv_v = [wb[:, 512 + 256 * i:512 + 256 * (i + 1)] for i in range(2)]
            bq_t = constp.tile([128, 1], F32, name="bq")

            # --- input DMAs: 4 large transfers (fixed DMA cost dominates
            # small ones). gpsimd SWDGE carries the residual so both HWDGE
            # queues are free for the weight blob + xf halves. ---
            nc.sync.dma_start(wb[:], wts[:, :])
            nc.sync.dma_start(xf_t[0][:], xf0[:, :])
            nc.scalar.dma_start(xf_t[1][:], xf1[:, :])
            nc.gpsimd.dma_start(xt[:], xts[:, :])
            # bq must be fp32 for the DVE scalar add: convert from the blob
            nc.vector.tensor_copy(bq_t[:], wb[:, 1024:1025])

            # PE warm-up weights: memset on GPSIMD so it clears during the
            # framework preamble; warm-up matmuls beat the HAM throttle while
            # the input DMAs stream.
            wz = constp.tile([128, 512], BF16, name="wz")
            nc.gpsimd.memset(wz[:], 0.0)
            for g in range(NG):
                nc.gpsimd.memset(vT_t[g][:, C::VW], 1.0)

            pswu = psp.tile([128, 512], F32, tag="ot3", name="pswu")
            for w in range(10):
                nc.tensor.matmul(pswu[:], lhsT=wz[:, 0:128], rhs=wz[:],
                                 start=True, stop=True, skip_group_check=True)

            # phase-1 PSUM ring over the phase-2 tags
            _ring = ["ot0", "ot1", "ot2", "ea", "ed"]
            _rix = [0]

            def p1tile(nm):
                t = psp.tile([128, 512], F32, tag=_ring[_rix[0] % len(_ring)],
                             name=nm)
                _rix[0] += 1
                return t

            # --- phase 1: projections per 512-key chunk ---
            for c in range(NG):
                psk = p1tile(f"psk{c}")
                for cb in range(2):
                    nc.tensor.matmul(
                        psk[:], lhsT=wk_v[cb], rhs=xf_t[cb][:, ts(c, 512)],
                        start=(cb == 0), stop=(cb == 1), skip_group_check=True)
                nc.scalar.copy(kT_t[c][:], psk[:])
                if c < NCP:
                    psq = p1tile(f"psq{c}")
                    for cb in range(2):
                        nc.tensor.matmul(
                            psq[:], lhsT=wq_v[cb], rhs=xf_t[cb][:, ts(c, 512)],
                            start=(cb == 0), stop=(cb == 1),
                            skip_group_check=True)
                    nc.vector.tensor_scalar_add(qT_t[c][:], psq[:],
                                                bq_t[:, 0:1])
                for rp in range(2):
                    p = 2 * c + rp
                    psv = p1tile(f"psv{p}")
                    for rr in range(2):
                        r = 2 * rp + rr
                        for cb in range(2):
                            nc.tensor.matmul(
                                psv[:, ts(rr, C)],
                                lhsT=xf_t[cb][:, 512 * c + 128 * r:
                                              512 * c + 128 * (r + 1)],
                                rhs=wv_v[cb],
                                start=(cb == 0), stop=(cb == 1),
                                skip_group_check=True)
                    dst = vT_t[c][:, 2 * rp * VW:(2 * rp + 2) * VW].rearrange(
                        "p (b w) -> p b w", w=VW)[:, :, 0:C]
                    srcap = psv[:].rearrange("p (b w) -> p b w", w=C)
                    if p % 8 < 3:
                        nc.scalar.copy(dst, srcap)
                    else:
                        nc.vector.tensor_copy(dst, srcap)

            # --- phase 2: 32 pairs of units; even unit exp on ACT, odd on
            # DVE (Schraudolph). O matmuls lag one pair. ---
            def emit_o(pend):
                aA, aB, pk, pcp, pot = pend
                for half, a in ((0, aA), (1, aB)):
                    for r in range(2):
                        m = 4 * pk + 2 * half + r
                        st, sp = (m == 0), (m == MB - 1)
                        gv = vT_t[m // 4][:, (m % 4) * VW:(m % 4) * VW + C + 1]
                        for j in range(4):
                            nc.tensor.matmul(
                                pot[j][:],
                                lhsT=a[:, 512 * r + 128 * j:
                                       512 * r + 128 * (j + 1)],
                                rhs=gv,
                                start=st, stop=sp, skip_group_check=True)
                if pk == 7:
                    fo = finp.tile([128, 1024], BF16, tag="fo",
                                   name=f"fo{pcp}")
                    for j in range(4):
                        sl = 4 * pcp + j
                        rcp = finp.tile([128, 1], F32, tag="r", bufs=4,
                                        name=f"r{pcp}_{j}")
                        nc.vector.reciprocal(rcp[:], pot[j][:, C:C + 1])
                        t = finp.tile([128, C], F32, tag="t", bufs=4,
                                      name=f"t{pcp}_{j}")
                        if j % 2 == 0:
                            nc.scalar.activation(t[:], pot[j][:, 0:C],
                                                 AF.Copy, scale=rcp[:, 0:1])
                        else:
                            nc.vector.tensor_scalar_mul(t[:], pot[j][:, 0:C],
                                                        rcp[:, 0:1])
                        nc.vector.tensor_tensor(fo[:, ts(j, C)], t[:],
                                                xt[:, ts(sl, C)], OP.add)
                    oeng = nc.sync if pcp % 2 == 0 else nc.scalar
                    oeng.dma_start(outS[:, ts(pcp, 1024)], fo[:])

            pend = None
            for cp in range(NCP):
                ot = [psp.tile([128, C + 1], F32, tag=f"ot{j}",
                               name=f"ot{j}_{cp}") for j in range(4)]
                for k in range(8):
                    eA = psp.tile([128, 1024], F32, tag="ea", name=f"eA{cp}_{k}")
                    eB = psp.tile([128, 1024], F32, tag="ed", name=f"eB{cp}_{k}")
                    for t4, (e, half, r) in enumerate(
                            ((eA, 0, 0), (eA, 0, 1), (eB, 1, 0), (eB, 1, 1))):
                        m = 4 * k + 2 * half + r
                        nc.tensor.matmul(
                            e[:, ts(r, 512)],
                            lhsT=kT_t[m // 4][32 * t4:32 * (t4 + 1),
                                              128 * (m % 4):128 * (m % 4) + 128],
                            rhs=qT_t[cp][32 * t4:32 * (t4 + 1), :],
                            start=True, stop=True, skip_group_check=True,
                            tile_position=(32 * t4, 0),
                        )
                    aA = apool.tile([128, 1024], BF16, tag="a", name=f"aA{cp}_{k}")
                    nc.scalar.activation(aA[:], eA[:], AF.Exp)
                    aB = apool.tile([128, 1024], BF16, tag="a", name=f"aB{cp}_{k}")
                    nc.vector.tensor_scalar(aB[:].bitcast(I16), eB[:],
                                            SCH_SCALE, SCH_BIAS,
                                            OP.mult, OP.add)
                    if pend is not None:
                        emit_o(pend)
                    pend = (aA, aB, k, cp, ot)
            emit_o(pend)
    _strip_self_waits(nc)
    _strip_redundant_mm_incs(nc)
    _split_multi_waits(nc)
    return nc


_ENGINE_SEM_PREFIX = {
    "EngineType.PE": "PE_",
    "EngineType.DVE": "DVE_",
    "EngineType.Activation": "Activation_",
    "EngineType.Pool": "Pool_",
    "EngineType.SP": "SP_",
}


def _strip_self_waits(nc):
    """Drop same-engine semaphore waits from multi-wait TPB instructions.

    Walrus allows exactly one sync wait per TPB instruction. Tile emits
    redundant self-engine waits (WAW on pool-slot reuse, RAW from same-engine
    producers): each engine executes its queue in order, so a wait on the
    engine's own semaphore is always satisfied by program order.
    """
    for bb in nc.m.functions[0].blocks:
        for inst in bb.instructions:
            si = inst.sync_info
            if si is None:
                continue
            w = si.on_wait
            if len(w) <= 1 or inst.opcode == "Drain":
                continue
            pfx = _ENGINE_SEM_PREFIX.get(str(inst.engine))
            if pfx is None:
                continue
            kept = [x for x in w if not x.ant_name.startswith(pfx)]
            if kept and len(kept) < len(w):
                si.on_wait = kept


def _strip_redundant_mm_incs(nc):
    """Drop per-matmul semaphore increments that no wait references.

    Tile emits `then_inc(PE_sem, 1)` on every matmul; each inc serializes
    ~26ns on the PE (EVT_SEM register write). Matmuls complete in pc order,
    so an inc is only needed at cumulative positions some wait references.
    Keep those, strip the rest, and remap every wait threshold to the new
    cumulative numbering.
    """
    from collections import defaultdict

    refd = defaultdict(set)
    for bb in nc.m.functions[0].blocks:
        for inst in bb.instructions:
            si = inst.sync_info
            if si is None:
                continue
            for w in si.on_wait:
                if w.wait_value is not None:
                    refd[w.ant_name].add(w.wait_value)

    sem_count = defaultdict(int)
    kept_count = defaultdict(int)
    remap = {}
    for bb in nc.m.functions[0].blocks:
        for inst in bb.instructions:
            if inst.opcode != "Matmult":
                continue
            si = inst.sync_info
            if si is None or not si.on_update:
                continue
            keep = []
            for u in si.on_update:
                s = u.ant_name
                if not s.startswith("PE_") or u.update_value != 1:
                    keep.append(u)
                    continue
                sem_count[s] += 1
                i = sem_count[s]
                if i in refd[s]:
                    kept_count[s] += 1
                    remap.setdefault(s, {})[i] = kept_count[s]
                    keep.append(u)
            si.on_update = keep

    for bb in nc.m.functions[0].blocks:
        for inst in bb.instructions:
            si = inst.sync_info
            if si is None:
                continue
            for w in si.on_wait:
                s = w.ant_name
                if s in remap and w.wait_value in remap[s]:
                    w.wait_value = remap[s][w.wait_value]


def _split_multi_waits(nc):
    """Walrus allows one sync wait per TPB instruction; move surplus waits
    onto dedicated single-wait Drain instructions inserted just before the
    offender (same engine, executes in order)."""
    import bass_rust
    cnt = 0
    for bb in nc.m.functions[0].blocks:
        il = bb.instructions
        i = 0
        while i < len(il):
            inst = il[i]
            si = inst.sync_info
            w = si.on_wait if si else []
            if len(w) > 1:
                for j, wait in enumerate(w[:-1]):
                    d = mybir.InstDrain(name=f"{inst.name}-w{j}", ins=[], outs=[],
                                        bass_is_fusable=False)
                    d.engine = inst.engine
                    d.sync_info = bass_rust.SyncInfo(on_wait=[wait], on_update=[])
                    il.insert(i, d)
                    i += 1
                    cnt += 1
                si.on_wait = [w[-1]]
            i += 1
    return cnt


_NC_CACHE = None


def _get_nc():
    global _NC_CACHE
    if _NC_CACHE is None:
        _NC_CACHE = _build()
    return _NC_CACHE


def kernel(x, wq, bq, wk, bk, wv, bv, gamma, _trace=False):
    f32 = lambda a: np.ascontiguousarray(np.asarray(a, dtype=np.float32))
    bf16 = lambda a: np.ascontiguousarray(np.asarray(a, dtype=np.float32)
                                          .astype(ml_dtypes.bfloat16))
    x = f32(x)
    g = float(np.asarray(gamma).reshape(-1)[0])
    xfull = x.reshape(B, C, N)

    wq4 = np.tile(np.asarray(wq, dtype=np.float32).T, (1, 4))   # [C, 128]
    wk4 = np.tile(np.asarray(wk, dtype=np.float32).T, (1, 4))
    wvT = (g * np.asarray(wv, dtype=np.float32)).T              # [C, C]
    bq4 = np.tile(np.asarray(bq, dtype=np.float32).reshape(D, 1),
                  (128 // D, 1))                                 # [128, 1]
    blob = np.zeros((128, WBLOB), np.float32)
    blob[:, 0:128] = wq4[0:128]
    blob[:, 128:256] = wq4[128:256]
    blob[:, 256:384] = wk4[0:128]
    blob[:, 384:512] = wk4[128:256]
    blob[:, 512:768] = wvT[0:128]
    blob[:, 768:1024] = wvT[128:256]
    blob[:, 1024:1025] = bq4
    shared = {"wts": bf16(blob)}

    gbv_row = (g * np.asarray(bv, dtype=np.float32)).reshape(1, C)
    in_maps = []
    for core in range(NCORES):
        b, h = core // 2, core % 2
        m = dict(shared)
        if h == 0:
            xr = xfull[b]
        else:
            # rotate so this core's query half sits at columns 0..NQ-1;
            # key order is irrelevant (attention reduces over all keys)
            xr = np.concatenate([xfull[b][:, NQ:], xfull[b][:, :NQ]], axis=1)
        m["xf0"] = bf16(xr[0:128])
        m["xf1"] = bf16(xr[128:256])
        xtq = xr[:, :NQ].T + gbv_row                              # [NQ, C]
        m["xts"] = bf16(xtq.reshape(16, 128, C).transpose(1, 0, 2)
                        .reshape(128, 16 * C))
        in_maps.append(m)

    res = run_bass_kernel_spmd(_get_nc(), in_maps, list(range(NCORES)),
                               trace=_trace)
    full = np.empty((B, C, N), np.float32)
    for core in range(NCORES):
        b, h = core // 2, core % 2
        o = np.asarray(res.results[core]["outS"], dtype=np.float32)
        # outS[p, cp*1024 + j*256 + c] = out^T[cp*512 + j*128 + p, c]
        oT = o.reshape(128, NCP * 4, C).transpose(1, 0, 2).reshape(NQ, C)
        full[b][:, h * NQ:(h + 1) * NQ] = oT.T
    out = full.reshape(B, C, HH, WW)
    if _trace:
        return out, res
    return out


# revision 8
# speedup vs baseline: 1.2901x; 1.0190x over previous
"""Spatial self-attention (SAGAN-style) kernel for 8 Trainium2 NeuronCores.

Math (per batch b):
    xf  = x[b].reshape(C, N)                       # C=256, N=4096
    qT  = wq @ xf + bq                             # [32, N]
    kT  = wk @ xf                                  # [32, N]  (bk dropped: per-query
                                                   #  const shift cancels in softmax)
    V0  = g*wv @ xf                                # [C, N]   (g*bv folded into
                                                   #  the residual on host)
    E^T = kT.T @ qT                                # [keys, queries]
    A'  = exp(E^T)          (no max-subtraction: |E| < 29, safe in fp32)
    s   = colsum(A')                               # softmax denominator
    out = (V0 @ A / s) + (x + g*bv)

Sharding: core i handles batch b = i//2, query half h = i%2 (2048 queries).
The host rotates xf per-core so the core's 2048 query columns sit at
columns 0..2047 (attention reductions are permutation-invariant over keys).

Device pipeline (v2):
  - Inputs ship as 4 large DMAs with 2-8KB/partition rows (per-DMA fixed
    cost ~1.4us dominates small transfers): a [128,1026] weight blob
    (wq|wk|wv halves + bq column, all bf16), xf as two [128,4096] halves
    on the two HWDGE queues, and the pre-swizzled residual x^T+g*bv as
    one [128,4096] SWDGE transfer on the gpsimd queue.
  - Phase 1: per 512-key chunk: qT/kT projections (wq/wk shipped 4x
    column-replicated so the d-dim is pre-broadcast across the four
    32-row groups) and V^T blocks with a free ones column (VW stride).
    Extraction split between ACT and DVE.
  - Phase 2: 64 units u=(cp, g): cp = 512-query chunk, g = 2-key-block
    group. Units run in PAIRS (g even/odd) with SEPARATE PSUM e-tiles
    ("ea"/"ed", [128,1024] = 2 banks each): one 4-way row-packed E
    foursome fills both units' e-tiles (4 distinct PSUM banks), then
    the EVEN unit's exp runs on ACT (table exp) while the ODD unit's
    exp runs CONCURRENTLY on DVE as a Schraudolph bit-trick:
        bf16_bits(exp(x)) ~= int16(x * 128*log2(e) + 127*128 - c)
    (one tensor_scalar mult+add, round-to-nearest f32->int16, written
    through a .bitcast(int16) view of the bf16 a-tile; max rel err
    ~3.3% at c=5.5 -- washes out to <1e-3 in the final output).
    With the two exp engines alternating, the PE never waits on a
    single-buffered e-tile and runs O matmuls back-to-back.
  - O^T accumulated in four [128,257] PSUM tiles per cp; the ones
    column yields the softmax denominator for free. Emission per cp
    writes scaled+residual-added slices into a [128,1024] staging tile
    shipped as ONE output DMA per cp (host unswizzles).
  - PSUM budget: ea(2) + ed(2) + ot0..ot3(4) = 8 banks; phase-1
    projection PSUMs ride a ring over the same tags.
  - Post-processing: walrus allows one semaphore wait per TPB
    instruction (_strip_self_waits, _split_multi_waits), and Tile's
    per-matmul then_inc costs ~26ns each on the PE -- all increments
    at cumulative positions no wait references are stripped and the
    remaining thresholds remapped (_strip_redundant_mm_incs).
"""

import ml_dtypes
import numpy as np

import concourse.bass as bass
import concourse.mybir as mybir
import concourse.tile as tile
from concourse.bass import ts
from concourse.bass_utils import run_bass_kernel_spmd

B, C, HH, WW = 4, 256, 64, 64
N = HH * WW          # 4096 spatial positions
D = 32               # C // 8 head dim
NCORES = 8
NQ = N * B // NCORES  # 2048 queries per core
MB = N // 128        # 32 key blocks
NCP = NQ // 512      # 4 query chunks of 512 per core
NG = N // 512        # 8 vT key groups of 4 blocks
NU = N // 256        # 16 units of 2 key blocks per chunk

F32 = mybir.dt.float32
BF16 = mybir.dt.bfloat16
I16 = mybir.dt.int16
AF = mybir.ActivationFunctionType
OP = mybir.AluOpType

VW = C + 2          # vT block width: 256 channels + ones col + pad
WBLOB = 4 * 128 + 2 * 256 + 2   # wq0|wq1|wk0|wk1|wv0|wv1|bq|pad

# Schraudolph constants: int16(E * 128*log2e + (127*128 - c)) viewed as bf16
SCH_SCALE = 1.4426950408889634 * 128.0
SCH_BIAS = 127.0 * 128.0 - 5.5


def _build():
    nc = bass.Bass()
    wts = nc.declare_dram_parameter("wts", [128, WBLOB], BF16, isOutput=False)
    xf0 = nc.declare_dram_parameter("xf0", [128, N], BF16, isOutput=False)
    xf1 = nc.declare_dram_parameter("xf1", [128, N], BF16, isOutput=False)
    xts = nc.declare_dram_parameter("xts", [128, 16 * C], BF16, isOutput=False)
    outS = nc.declare_dram_parameter("outS", [128, NCP * 1024], BF16,
                                     isOutput=True)

    with tile.TileContext(nc) as tc:
        with (
            tc.tile_pool(name="const", bufs=1) as constp,
            tc.tile_pool(name="big", bufs=1) as bigp,
            tc.tile_pool(name="apool", bufs=6) as apool,
            tc.tile_pool(name="fin", bufs=2) as finp,
            tc.tile_pool(name="ps", bufs=1, space="PSUM") as psp,
        ):
            wb = constp.tile([128, WBLOB], BF16, name="wb")
            xf_t = [[constp.tile([128, N // 2], BF16, name=f"xf{i}{p}")
                     for p in range(2)] for i in range(2)]
            kT_t = [bigp.tile([128, 512], BF16, name=f"kT{c}") for c in range(NG)]
            qT_t = [bigp.tile([128, 512], BF16, name=f"qT{c}") for c in range(NCP)]
            vT_t = [bigp.tile([128, 4 * VW], BF16, name=f"vT{g}") for g in range(NG)]
            xt = bigp.tile([128, 16 * C], BF16, name="xt")

            wq_v = [wb[:, 128 * i:128 * (i + 1)] for i in range(2)]
            wk_v = [wb[:, 256 + 128 * i:256 + 128 * (i + 1)] for i in range(2)]
            wv_v = [wb[:, 512 + 256 * i:512 + 256 * (i + 1)] for i in range(2)]
            bq_t = constp.tile([128, 1], F32, name="bq")

            # --- input DMAs: 4 large transfers (fixed DMA cost dominates
            # small ones). gpsimd SWDGE carries the residual so both HWDGE
            # queues are free for the weight blob + xf halves. ---
            nc.gpsimd.dma_start(wb[:], wts[:, :])
            for p in range(2):
                nc.sync.dma_start(xf_t[0][p][:], xf0[:, ts(p, N // 2)])
                nc.scalar.dma_start(xf_t[1][p][:], xf1[:, ts(p, N // 2)])
            nc.gpsimd.dma_start(xt[:], xts[:, :])
            # bq must be fp32 for the DVE scalar add: convert from the blob
            nc.vector.tensor_copy(bq_t[:], wb[:, 1024:1025])

            # PE warm-up weights: memset on GPSIMD so it clears during the
            # framework preamble; warm-up matmuls beat the HAM throttle while
            # the input DMAs stream.
            wz = constp.tile([128, 512], BF16, name="wz")
            nc.gpsimd.memset(wz[:], 0.0)
            for g in range(NG):
                nc.gpsimd.memset(vT_t[g][:, C::VW], 1.0)

            pswu = psp.tile([128, 512], F32, tag="ot3", name="pswu")
            for w in range(10):
                nc.tensor.matmul(pswu[:], lhsT=wz[:, 0:128], rhs=wz[:],
                                 start=True, stop=True, skip_group_check=True)

            # phase-1 PSUM ring over the phase-2 tags
            _ring = ["ot0", "ot1", "ot2", "ea", "ed"]
            _rix = [0]

            def p1tile(nm):
                t = psp.tile([128, 512], F32, tag=_ring[_rix[0] % len(_ring)],
                             name=nm)
                _rix[0] += 1
                return t

            # --- phase 1: projections per 512-key chunk ---
            for c in range(NG):
                psk = p1tile(f"psk{c}")
                for cb in range(2):
                    nc.tensor.matmul(
                        psk[:], lhsT=wk_v[cb],
                        rhs=xf_t[cb][c // 4][:, ts(c % 4, 512)],
                        start=(cb == 0), stop=(cb == 1), skip_group_check=True)
                nc.scalar.copy(kT_t[c][:], psk[:])
                if c < NCP:
                    psq = p1tile(f"psq{c}")
                    for cb in range(2):
                        nc.tensor.matmul(
                            psq[:], lhsT=wq_v[cb],
                            rhs=xf_t[cb][c // 4][:, ts(c % 4, 512)],
                            start=(cb == 0), stop=(cb == 1),
                            skip_group_check=True)
                    nc.vector.tensor_scalar_add(qT_t[c][:], psq[:],
                                                bq_t[:, 0:1])
                for rp in range(2):
                    p = 2 * c + rp
                    psv = p1tile(f"psv{p}")
                    for rr in range(2):
                        r = 2 * rp + rr
                        for cb in range(2):
                            nc.tensor.matmul(
                                psv[:, ts(rr, C)],
                                lhsT=xf_t[cb][c // 4][
                                    :, 512 * (c % 4) + 128 * r:
                                    512 * (c % 4) + 128 * (r + 1)],
                                rhs=wv_v[cb],
                                start=(cb == 0), stop=(cb == 1),
                                skip_group_check=True)
                    dst = vT_t[c][:, 2 * rp * VW:(2 * rp + 2) * VW].rearrange(
                        "p (b w) -> p b w", w=VW)[:, :, 0:C]
                    srcap = psv[:].rearrange("p (b w) -> p b w", w=C)
                    if p % 8 < 3:
                        nc.scalar.copy(dst, srcap)
                    else:
                        nc.vector.tensor_copy(dst, srcap)

            # --- phase 2: 32 pairs of units; even unit exp on ACT, odd on
            # DVE (Schraudolph). O matmuls lag one pair. ---
            def emit_o(pend):
                aA, aB, pk, pcp, pot = pend
                for half, a in ((0, aA), (1, aB)):
                    for r in range(2):
                        m = 4 * pk + 2 * half + r
                        st, sp = (m == 0), (m == MB - 1)
                        gv = vT_t[m // 4][:, (m % 4) * VW:(m % 4) * VW + C + 1]
                        for j in range(4):
                            nc.tensor.matmul(
                                pot[j][:],
                                lhsT=a[:, 512 * r + 128 * j:
                                       512 * r + 128 * (j + 1)],
                                rhs=gv,
                                start=st, stop=sp, skip_group_check=True)
                if pk == 7:
                    fo = finp.tile([128, 1024], BF16, tag="fo",
                                   name=f"fo{pcp}")
                    for j in range(4):
                        sl = 4 * pcp + j
                        rcp = finp.tile([128, 1], F32, tag="r", bufs=4,
                                        name=f"r{pcp}_{j}")
                        nc.vector.reciprocal(rcp[:], pot[j][:, C:C + 1])
                        t = finp.tile([128, C], F32, tag="t", bufs=4,
                                      name=f"t{pcp}_{j}")
                        if j % 2 == 0:
                            nc.scalar.activation(t[:], pot[j][:, 0:C],
                                                 AF.Copy, scale=rcp[:, 0:1])
                        else:
                            nc.vector.tensor_scalar_mul(t[:], pot[j][:, 0:C],
                                                        rcp[:, 0:1])
                        nc.vector.tensor_tensor(fo[:, ts(j, C)], t[:],
                                                xt[:, ts(sl, C)], OP.add)
                    for hh in range(2):
                        oeng = nc.sync if (2 * pcp + hh) % 2 == 0 else nc.scalar
                        oeng.dma_start(
                            outS[:, 1024 * pcp + 512 * hh:
                                 1024 * pcp + 512 * (hh + 1)],
                            fo[:, ts(hh, 512)])

            pend = None
            for cp in range(NCP):
                ot = [psp.tile([128, C + 1], F32, tag=f"ot{j}",
                               name=f"ot{j}_{cp}") for j in range(4)]
                for k in range(8):
                    eA = psp.tile([128, 1024], F32, tag="ea", name=f"eA{cp}_{k}")
                    eB = psp.tile([128, 1024], F32, tag="ed", name=f"eB{cp}_{k}")
                    for t4, (e, half, r) in enumerate(
                            ((eA, 0, 0), (eA, 0, 1), (eB, 1, 0), (eB, 1, 1))):
                        m = 4 * k + 2 * half + r
                        nc.tensor.matmul(
                            e[:, ts(r, 512)],
                            lhsT=kT_t[m // 4][32 * t4:32 * (t4 + 1),
                                              128 * (m % 4):128 * (m % 4) + 128],
                            rhs=qT_t[cp][32 * t4:32 * (t4 + 1), :],
                            start=True, stop=True, skip_group_check=True,
                            tile_position=(32 * t4, 0),
                        )
                    aA = apool.tile([128, 1024], BF16, tag="a", name=f"aA{cp}_{k}")
                    nc.scalar.activation(aA[:], eA[:], AF.Exp)
                    aB = apool.tile([128, 1024], BF16, tag="a", name=f"aB{cp}_{k}")
                    nc.vector.tensor_scalar(aB[:].bitcast(I16), eB[:],
                                            SCH_SCALE, SCH_BIAS,
                                            OP.mult, OP.add)
                    if pend is not None:
                        emit_o(pend)
                    pend = (aA, aB, k, cp, ot)
            emit_o(pend)
    _strip_self_waits(nc)
    _strip_redundant_mm_incs(nc)
    _split_multi_waits(nc)
    return nc


_ENGINE_SEM_PREFIX = {
    "EngineType.PE": "PE_",
    "EngineType.DVE": "DVE_",
    "EngineType.Activation": "Activation_",
    "EngineType.Pool": "Pool_",
    "EngineType.SP": "SP_",
}


def _strip_self_waits(nc):
    """Drop same-engine semaphore waits from multi-wait TPB instructions.

    Walrus allows exactly one sync wait per TPB instruction. Tile emits
    redundant self-engine waits (WAW on pool-slot reuse, RAW from same-engine
    producers): each engine executes its queue in order, so a wait on the
    engine's own semaphore is always satisfied by program order.
    """
    for bb in nc.m.functions[0].blocks:
        for inst in bb.instructions:
            si = inst.sync_info
            if si is None:
                continue
            w = si.on_wait
            if len(w) <= 1 or inst.opcode == "Drain":
                continue
            pfx = _ENGINE_SEM_PREFIX.get(str(inst.engine))
            if pfx is None:
                continue
            kept = [x for x in w if not x.ant_name.startswith(pfx)]
            if kept and len(kept) < len(w):
                si.on_wait = kept


def _strip_redundant_mm_incs(nc):
    """Drop per-matmul semaphore increments that no wait references.

    Tile emits `then_inc(PE_sem, 1)` on every matmul; each inc serializes
    ~26ns on the PE (EVT_SEM register write). Matmuls complete in pc order,
    so an inc is only needed at cumulative positions some wait references.
    Keep those, strip the rest, and remap every wait threshold to the new
    cumulative numbering.
    """
    from collections import defaultdict

    refd = defaultdict(set)
    for bb in nc.m.functions[0].blocks:
        for inst in bb.instructions:
            si = inst.sync_info
            if si is None:
                continue
            for w in si.on_wait:
                if w.wait_value is not None:
                    refd[w.ant_name].add(w.wait_value)

    sem_count = defaultdict(int)
    kept_count = defaultdict(int)
    remap = {}
    for bb in nc.m.functions[0].blocks:
        for inst in bb.instructions:
            if inst.opcode != "Matmult":
                continue
            si = inst.sync_info
            if si is None or not si.on_update:
                continue
            keep = []
            for u in si.on_update:
                s = u.ant_name
                if not s.startswith("PE_") or u.update_value != 1:
                    keep.append(u)
                    continue
                sem_count[s] += 1
                i = sem_count[s]
                if i in refd[s]:
                    kept_count[s] += 1
                    remap.setdefault(s, {})[i] = kept_count[s]
                    keep.append(u)
            si.on_update = keep

    for bb in nc.m.functions[0].blocks:
        for inst in bb.instructions:
            si = inst.sync_info
            if si is None:
                continue
            for w in si.on_wait:
                s = w.ant_name
                if s in remap and w.wait_value in remap[s]:
                    w.wait_value = remap[s][w.wait_value]


def _split_multi_waits(nc):
    """Walrus allows one sync wait per TPB instruction; move surplus waits
    onto dedicated single-wait Drain instructions inserted just before the
    offender (same engine, executes in order)."""
    import bass_rust
    cnt = 0
    for bb in nc.m.functions[0].blocks:
        il = bb.instructions
        i = 0
        while i < len(il):
            inst = il[i]
            si = inst.sync_info
            w = si.on_wait if si else []
            if len(w) > 1:
                for j, wait in enumerate(w[:-1]):
                    d = mybir.InstDrain(name=f"{inst.name}-w{j}", ins=[], outs=[],
                                        bass_is_fusable=False)
                    d.engine = inst.engine
                    d.sync_info = bass_rust.SyncInfo(on_wait=[wait], on_update=[])
                    il.insert(i, d)
                    i += 1
                    cnt += 1
                si.on_wait = [w[-1]]
            i += 1
    return cnt


_NC_CACHE = None


def _get_nc():
    global _NC_CACHE
    if _NC_CACHE is None:
        _NC_CACHE = _build()
    return _NC_CACHE


def kernel(x, wq, bq, wk, bk, wv, bv, gamma, _trace=False):
    f32 = lambda a: np.ascontiguousarray(np.asarray(a, dtype=np.float32))
    bf16 = lambda a: np.ascontiguousarray(np.asarray(a, dtype=np.float32)
                                          .astype(ml_dtypes.bfloat16))
    x = f32(x)
    g = float(np.asarray(gamma).reshape(-1)[0])
    xfull = x.reshape(B, C, N)

    wq4 = np.tile(np.asarray(wq, dtype=np.float32).T, (1, 4))   # [C, 128]
    wk4 = np.tile(np.asarray(wk, dtype=np.float32).T, (1, 4))
    wvT = (g * np.asarray(wv, dtype=np.float32)).T              # [C, C]
    bq4 = np.tile(np.asarray(bq, dtype=np.float32).reshape(D, 1),
                  (128 // D, 1))                                 # [128, 1]
    blob = np.zeros((128, WBLOB), np.float32)
    blob[:, 0:128] = wq4[0:128]
    blob[:, 128:256] = wq4[128:256]
    blob[:, 256:384] = wk4[0:128]
    blob[:, 384:512] = wk4[128:256]
    blob[:, 512:768] = wvT[0:128]
    blob[:, 768:1024] = wvT[128:256]
    blob[:, 1024:1025] = bq4
    shared = {"wts": bf16(blob)}

    gbv_row = (g * np.asarray(bv, dtype=np.float32)).reshape(1, C)
    in_maps = []
    for core in range(NCORES):
        b, h = core // 2, core % 2
        m = dict(shared)
        if h == 0:
            xr = xfull[b]
        else:
            # rotate so this core's query half sits at columns 0..NQ-1;
            # key order is irrelevant (attention reduces over all keys)
            xr = np.concatenate([xfull[b][:, NQ:], xfull[b][:, :NQ]], axis=1)
        m["xf0"] = bf16(xr[0:128])
        m["xf1"] = bf16(xr[128:256])
        xtq = xr[:, :NQ].T + gbv_row                              # [NQ, C]
        m["xts"] = bf16(xtq.reshape(16, 128, C).transpose(1, 0, 2)
                        .reshape(128, 16 * C))
        in_maps.append(m)

    res = run_bass_kernel_spmd(_get_nc(), in_maps, list(range(NCORES)),
                               trace=_trace)
    full = np.empty((B, C, N), np.float32)
    for core in range(NCORES):
        b, h = core // 2, core % 2
        o = np.asarray(res.results[core]["outS"], dtype=np.float32)
        # outS[p, cp*1024 + j*256 + c] = out^T[cp*512 + j*128 + p, c]
        oT = o.reshape(128, NCP * 4, C).transpose(1, 0, 2).reshape(NQ, C)
        full[b][:, h * NQ:(h + 1) * NQ] = oT.T
    out = full.reshape(B, C, HH, WW)
    if _trace:
        return out, res
    return out


# revision 9
# speedup vs baseline: 1.3278x; 1.0292x over previous
"""Spatial self-attention (SAGAN-style) kernel for 8 Trainium2 NeuronCores.

Math (per batch b):
    xf  = x[b].reshape(C, N)                       # C=256, N=4096
    qT  = wq @ xf + bq                             # [32, N]
    kT  = wk @ xf                                  # [32, N]  (bk dropped: per-query
                                                   #  const shift cancels in softmax)
    V0  = g*wv @ xf                                # [C, N]   (g*bv folded into
                                                   #  the residual on host)
    E^T = kT.T @ qT                                # [keys, queries]
    A'  = exp(E^T)          (no max-subtraction: |E| < 29, safe in fp32)
    s   = colsum(A')                               # softmax denominator
    out = (V0 @ A / s) + (x + g*bv)

Sharding: core i handles batch b = i//2, query half h = i%2 (2048 queries).
The host rotates xf per-core so the core's 2048 query columns sit at
columns 0..2047 (attention reductions are permutation-invariant over keys).

Device pipeline (v2):
  - Inputs ship as 4 large DMAs with 2-8KB/partition rows (per-DMA fixed
    cost ~1.4us dominates small transfers): a [128,1026] weight blob
    (wq|wk|wv halves + bq column, all bf16), xf as two [128,4096] halves
    on the two HWDGE queues, and the pre-swizzled residual x^T+g*bv as
    one [128,4096] SWDGE transfer on the gpsimd queue.
  - Phase 1: per 512-key chunk: qT/kT projections (wq/wk shipped 4x
    column-replicated so the d-dim is pre-broadcast across the four
    32-row groups) and V^T blocks with a free ones column (VW stride).
    Extraction split between ACT and DVE.
  - Phase 2: 64 units u=(cp, g): cp = 512-query chunk, g = 2-key-block
    group. Units run in PAIRS (g even/odd) with SEPARATE PSUM e-tiles
    ("ea"/"ed", [128,1024] = 2 banks each): one 4-way row-packed E
    foursome fills both units' e-tiles (4 distinct PSUM banks), then
    the EVEN unit's exp runs on ACT (table exp) while the ODD unit's
    exp runs CONCURRENTLY on DVE as a Schraudolph bit-trick:
        bf16_bits(exp(x)) ~= int16(x * 128*log2(e) + 127*128 - c)
    (one tensor_scalar mult+add, round-to-nearest f32->int16, written
    through a .bitcast(int16) view of the bf16 a-tile; max rel err
    ~3.3% at c=5.5 -- washes out to <1e-3 in the final output).
    With the two exp engines alternating, the PE never waits on a
    single-buffered e-tile and runs O matmuls back-to-back.
  - O^T accumulated in four [128,257] PSUM tiles per cp; the ones
    column yields the softmax denominator for free. Emission per cp
    writes scaled+residual-added slices into a [128,1024] staging tile
    shipped as ONE output DMA per cp (host unswizzles).
  - PSUM budget: ea(2) + ed(2) + ot0..ot3(4) = 8 banks; phase-1
    projection PSUMs ride a ring over the same tags.
  - Post-processing: walrus allows one semaphore wait per TPB
    instruction (_strip_self_waits, _split_multi_waits), and Tile's
    per-matmul then_inc costs ~26ns each on the PE -- all increments
    at cumulative positions no wait references are stripped and the
    remaining thresholds remapped (_strip_redundant_mm_incs).
"""

import ml_dtypes
import numpy as np

import concourse.bass as bass
import concourse.mybir as mybir
import concourse.tile as tile
from concourse.bass import ts
from concourse.bass_utils import run_bass_kernel_spmd

B, C, HH, WW = 4, 256, 64, 64
N = HH * WW          # 4096 spatial positions
D = 32               # C // 8 head dim
NCORES = 8
NQ = N * B // NCORES  # 2048 queries per core
MB = N // 128        # 32 key blocks
NCP = NQ // 512      # 4 query chunks of 512 per core
NG = N // 512        # 8 vT key groups of 4 blocks
NU = N // 256        # 16 units of 2 key blocks per chunk

F32 = mybir.dt.float32
BF16 = mybir.dt.bfloat16
I16 = mybir.dt.int16
AF = mybir.ActivationFunctionType
OP = mybir.AluOpType

VW = C + 2          # vT block width: 256 channels + ones col + pad
WBLOB = 4 * 128 + 2 * 256 + 2   # wq0|wq1|wk0|wk1|wv0|wv1|bq|pad

# Schraudolph constants: int16(E * 128*log2e + (127*128 - c)) viewed as bf16
SCH_SCALE = 1.4426950408889634 * 128.0
SCH_BIAS = 127.0 * 128.0 - 5.5


def _build():
    nc = bass.Bass()
    wts = nc.declare_dram_parameter("wts", [128, WBLOB], BF16, isOutput=False)
    xf0 = nc.declare_dram_parameter("xf0", [128, N], BF16, isOutput=False)
    xf1 = nc.declare_dram_parameter("xf1", [128, N], BF16, isOutput=False)
    xts = nc.declare_dram_parameter("xts", [128, 16 * C], BF16, isOutput=False)
    outS = nc.declare_dram_parameter("outS", [128, NCP * 1024], BF16,
                                     isOutput=True)

    with tile.TileContext(nc) as tc:
        with (
            tc.tile_pool(name="const", bufs=1) as constp,
            tc.tile_pool(name="big", bufs=1) as bigp,
            tc.tile_pool(name="apool", bufs=6) as apool,
            tc.tile_pool(name="fin", bufs=2) as finp,
            tc.tile_pool(name="ps", bufs=1, space="PSUM") as psp,
        ):
            wb = constp.tile([128, WBLOB], BF16, name="wb")
            XFW = [512, 512, 1024, 2048]
            XFO = [0, 512, 1024, 2048]
            xf_t = [[constp.tile([128, w], BF16, name=f"xf{i}{p}")
                     for p, w in enumerate(XFW)] for i in range(2)]

            def xfsl(cb, c):
                # chunk c of 512 cols -> (piece, col offset)
                p = 0 if c == 0 else 1 if c == 1 else 2 if c < 4 else 3
                return xf_t[cb][p][:, 512 * c - XFO[p]:512 * (c + 1) - XFO[p]]
            kT_t = [bigp.tile([128, 512], BF16, name=f"kT{c}") for c in range(NG)]
            qT_t = [bigp.tile([128, 512], BF16, name=f"qT{c}") for c in range(NCP)]
            vT_t = [bigp.tile([128, 4 * VW], BF16, name=f"vT{g}") for g in range(NG)]
            xt = bigp.tile([128, 16 * C], BF16, name="xt")

            wq_v = [wb[:, 128 * i:128 * (i + 1)] for i in range(2)]
            wk_v = [wb[:, 256 + 128 * i:256 + 128 * (i + 1)] for i in range(2)]
            wv_v = [wb[:, 512 + 256 * i:512 + 256 * (i + 1)] for i in range(2)]
            bq_t = constp.tile([128, 1], F32, name="bq")

            # --- input DMAs: 4 large transfers (fixed DMA cost dominates
            # small ones). gpsimd SWDGE carries the residual so both HWDGE
            # queues are free for the weight blob + xf halves. ---
            nc.gpsimd.dma_start(wb[:], wts[:, :])
            for p in range(4):
                nc.sync.dma_start(xf_t[0][p][:],
                                  xf0[:, XFO[p]:XFO[p] + XFW[p]])
                nc.scalar.dma_start(xf_t[1][p][:],
                                    xf1[:, XFO[p]:XFO[p] + XFW[p]])
            nc.gpsimd.dma_start(xt[:], xts[:, :])
            # bq must be fp32 for the DVE scalar add: convert from the blob
            nc.vector.tensor_copy(bq_t[:], wb[:, 1024:1025])

            # PE warm-up weights: memset on GPSIMD so it clears during the
            # framework preamble; warm-up matmuls beat the HAM throttle while
            # the input DMAs stream.
            wz = constp.tile([128, 512], BF16, name="wz")
            nc.gpsimd.memset(wz[:], 0.0)
            for g in range(NG):
                nc.gpsimd.memset(vT_t[g][:, C::VW], 1.0)

            pswu = psp.tile([128, 512], F32, tag="ot3", name="pswu")
            for w in range(10):
                nc.tensor.matmul(pswu[:], lhsT=wz[:, 0:128], rhs=wz[:],
                                 start=True, stop=True, skip_group_check=True)

            # phase-1 PSUM ring over the phase-2 tags
            _ring = ["ot0", "ot1", "ot2", "ea", "ed"]
            _rix = [0]

            def p1tile(nm):
                t = psp.tile([128, 512], F32, tag=_ring[_rix[0] % len(_ring)],
                             name=nm)
                _rix[0] += 1
                return t

            # --- phase 1: projections per 512-key chunk ---
            for c in range(NG):
                psk = p1tile(f"psk{c}")
                for cb in range(2):
                    nc.tensor.matmul(
                        psk[:], lhsT=wk_v[cb], rhs=xfsl(cb, c),
                        start=(cb == 0), stop=(cb == 1), skip_group_check=True)
                nc.scalar.copy(kT_t[c][:], psk[:])
                if c < NCP:
                    psq = p1tile(f"psq{c}")
                    for cb in range(2):
                        nc.tensor.matmul(
                            psq[:], lhsT=wq_v[cb], rhs=xfsl(cb, c),
                            start=(cb == 0), stop=(cb == 1),
                            skip_group_check=True)
                    nc.vector.tensor_scalar_add(qT_t[c][:], psq[:],
                                                bq_t[:, 0:1])
                for rp in range(2):
                    p = 2 * c + rp
                    psv = p1tile(f"psv{p}")
                    for rr in range(2):
                        r = 2 * rp + rr
                        for cb in range(2):
                            nc.tensor.matmul(
                                psv[:, ts(rr, C)],
                                lhsT=xfsl(cb, c)[:, 128 * r:128 * (r + 1)],
                                rhs=wv_v[cb],
                                start=(cb == 0), stop=(cb == 1),
                                skip_group_check=True)
                    dst = vT_t[c][:, 2 * rp * VW:(2 * rp + 2) * VW].rearrange(
                        "p (b w) -> p b w", w=VW)[:, :, 0:C]
                    srcap = psv[:].rearrange("p (b w) -> p b w", w=C)
                    if p % 8 < 3:
                        nc.scalar.copy(dst, srcap)
                    else:
                        nc.vector.tensor_copy(dst, srcap)

            # --- phase 2: 32 pairs of units; even unit exp on ACT, odd on
            # DVE (Schraudolph). O matmuls lag one pair. ---
            def emit_o(pend):
                aA, aB, pk, pcp, pot = pend
                for half, a in ((0, aA), (1, aB)):
                    for r in range(2):
                        m = 4 * pk + 2 * half + r
                        st, sp = (m == 0), (m == MB - 1)
                        gv = vT_t[m // 4][:, (m % 4) * VW:(m % 4) * VW + C + 1]
                        for j in range(4):
                            nc.tensor.matmul(
                                pot[j][:],
                                lhsT=a[:, 512 * r + 128 * j:
                                       512 * r + 128 * (j + 1)],
                                rhs=gv,
                                start=st, stop=sp, skip_group_check=True)
                if pk == 7:
                    fo = finp.tile([128, 1024], BF16, tag="fo",
                                   name=f"fo{pcp}")
                    for j in range(4):
                        sl = 4 * pcp + j
                        rcp = finp.tile([128, 1], F32, tag="r", bufs=4,
                                        name=f"r{pcp}_{j}")
                        nc.vector.reciprocal(rcp[:], pot[j][:, C:C + 1])
                        t = finp.tile([128, C], F32, tag="t", bufs=4,
                                      name=f"t{pcp}_{j}")
                        if j % 2 == 0:
                            nc.scalar.activation(t[:], pot[j][:, 0:C],
                                                 AF.Copy, scale=rcp[:, 0:1])
                        else:
                            nc.vector.tensor_scalar_mul(t[:], pot[j][:, 0:C],
                                                        rcp[:, 0:1])
                        nc.vector.tensor_tensor(fo[:, ts(j, C)], t[:],
                                                xt[:, ts(sl, C)], OP.add)
                    for hh in range(2):
                        oeng = nc.sync if (2 * pcp + hh) % 2 == 0 else nc.scalar
                        oeng.dma_start(
                            outS[:, 1024 * pcp + 512 * hh:
                                 1024 * pcp + 512 * (hh + 1)],
                            fo[:, ts(hh, 512)])

            pend = None
            for cp in range(NCP):
                ot = [psp.tile([128, C + 1], F32, tag=f"ot{j}",
                               name=f"ot{j}_{cp}") for j in range(4)]
                for k in range(8):
                    eA = psp.tile([128, 1024], F32, tag="ea", name=f"eA{cp}_{k}")
                    eB = psp.tile([128, 1024], F32, tag="ed", name=f"eB{cp}_{k}")
                    for t4, (e, half, r) in enumerate(
                            ((eA, 0, 0), (eA, 0, 1), (eB, 1, 0), (eB, 1, 1))):
                        m = 4 * k + 2 * half + r
                        nc.tensor.matmul(
                            e[:, ts(r, 512)],
                            lhsT=kT_t[m // 4][32 * t4:32 * (t4 + 1),
                                              128 * (m % 4):128 * (m % 4) + 128],
                            rhs=qT_t[cp][32 * t4:32 * (t4 + 1), :],
                            start=True, stop=True, skip_group_check=True,
                            tile_position=(32 * t4, 0),
                        )
                    aA = apool.tile([128, 1024], BF16, tag="a", name=f"aA{cp}_{k}")
                    nc.scalar.activation(aA[:], eA[:], AF.Exp)
                    aB = apool.tile([128, 1024], BF16, tag="a", name=f"aB{cp}_{k}")
                    nc.vector.tensor_scalar(aB[:].bitcast(I16), eB[:],
                                            SCH_SCALE, SCH_BIAS,
                                            OP.mult, OP.add)
                    if pend is not None:
                        emit_o(pend)
                    pend = (aA, aB, k, cp, ot)
            emit_o(pend)
    _strip_self_waits(nc)
    _strip_redundant_mm_incs(nc)
    _split_multi_waits(nc)
    return nc


_ENGINE_SEM_PREFIX = {
    "EngineType.PE": "PE_",
    "EngineType.DVE": "DVE_",
    "EngineType.Activation": "Activation_",
    "EngineType.Pool": "Pool_",
    "EngineType.SP": "SP_",
}


def _strip_self_waits(nc):
    """Drop same-engine semaphore waits from multi-wait TPB instructions.

    Walrus allows exactly one sync wait per TPB instruction. Tile emits
    redundant self-engine waits (WAW on pool-slot reuse, RAW from same-engine
    producers): each engine executes its queue in order, so a wait on the
    engine's own semaphore is always satisfied by program order.
    """
    for bb in nc.m.functions[0].blocks:
        for inst in bb.instructions:
            si = inst.sync_info
            if si is None:
                continue
            w = si.on_wait
            if len(w) <= 1 or inst.opcode == "Drain":
                continue
            pfx = _ENGINE_SEM_PREFIX.get(str(inst.engine))
            if pfx is None:
                continue
            kept = [x for x in w if not x.ant_name.startswith(pfx)]
            if kept and len(kept) < len(w):
                si.on_wait = kept


def _strip_redundant_mm_incs(nc):
    """Drop per-matmul semaphore increments that no wait references.

    Tile emits `then_inc(PE_sem, 1)` on every matmul; each inc serializes
    ~26ns on the PE (EVT_SEM register write). Matmuls complete in pc order,
    so an inc is only needed at cumulative positions some wait references.
    Keep those, strip the rest, and remap every wait threshold to the new
    cumulative numbering.
    """
    from collections import defaultdict

    refd = defaultdict(set)
    for bb in nc.m.functions[0].blocks:
        for inst in bb.instructions:
            si = inst.sync_info
            if si is None:
                continue
            for w in si.on_wait:
                if w.wait_value is not None:
                    refd[w.ant_name].add(w.wait_value)

    sem_count = defaultdict(int)
    kept_count = defaultdict(int)
    remap = {}
    for bb in nc.m.functions[0].blocks:
        for inst in bb.instructions:
            if inst.opcode != "Matmult":
                continue
            si = inst.sync_info
            if si is None or not si.on_update:
                continue
            keep = []
            for u in si.on_update:
                s = u.ant_name
                if not s.startswith("PE_") or u.update_value != 1:
                    keep.append(u)
                    continue
                sem_count[s] += 1
                i = sem_count[s]
                if i in refd[s]:
                    kept_count[s] += 1
                    remap.setdefault(s, {})[i] = kept_count[s]
                    keep.append(u)
            si.on_update = keep

    for bb in nc.m.functions[0].blocks:
        for inst in bb.instructions:
            si = inst.sync_info
            if si is None:
                continue
            for w in si.on_wait:
                s = w.ant_name
                if s in remap and w.wait_value in remap[s]:
                    w.wait_value = remap[s][w.wait_value]


def _split_multi_waits(nc):
    """Walrus allows one sync wait per TPB instruction; move surplus waits
    onto dedicated single-wait Drain instructions inserted just before the
    offender (same engine, executes in order)."""
    import bass_rust
    cnt = 0
    for bb in nc.m.functions[0].blocks:
        il = bb.instructions
        i = 0
        while i < len(il):
            inst = il[i]
            si = inst.sync_info
            w = si.on_wait if si else []
            if len(w) > 1:
                for j, wait in enumerate(w[:-1]):
                    d = mybir.InstDrain(name=f"{inst.name}-w{j}", ins=[], outs=[],
                                        bass_is_fusable=False)
                    d.engine = inst.engine
                    d.sync_info = bass_rust.SyncInfo(on_wait=[wait], on_update=[])
                    il.insert(i, d)
                    i += 1
                    cnt += 1
                si.on_wait = [w[-1]]
            i += 1
    return cnt


_NC_CACHE = None


def _get_nc():
    global _NC_CACHE
    if _NC_CACHE is None:
        _NC_CACHE = _build()
    return _NC_CACHE


def kernel(x, wq, bq, wk, bk, wv, bv, gamma, _trace=False):
    f32 = lambda a: np.ascontiguousarray(np.asarray(a, dtype=np.float32))
    bf16 = lambda a: np.ascontiguousarray(np.asarray(a, dtype=np.float32)
                                          .astype(ml_dtypes.bfloat16))
    x = f32(x)
    g = float(np.asarray(gamma).reshape(-1)[0])
    xfull = x.reshape(B, C, N)

    wq4 = np.tile(np.asarray(wq, dtype=np.float32).T, (1, 4))   # [C, 128]
    wk4 = np.tile(np.asarray(wk, dtype=np.float32).T, (1, 4))
    wvT = (g * np.asarray(wv, dtype=np.float32)).T              # [C, C]
    bq4 = np.tile(np.asarray(bq, dtype=np.float32).reshape(D, 1),
                  (128 // D, 1))                                 # [128, 1]
    blob = np.zeros((128, WBLOB), np.float32)
    blob[:, 0:128] = wq4[0:128]
    blob[:, 128:256] = wq4[128:256]
    blob[:, 256:384] = wk4[0:128]
    blob[:, 384:512] = wk4[128:256]
    blob[:, 512:768] = wvT[0:128]
    blob[:, 768:1024] = wvT[128:256]
    blob[:, 1024:1025] = bq4
    shared = {"wts": bf16(blob)}

    gbv_row = (g * np.asarray(bv, dtype=np.float32)).reshape(1, C)
    in_maps = []
    for core in range(NCORES):
        b, h = core // 2, core % 2
        m = dict(shared)
        if h == 0:
            xr = xfull[b]
        else:
            # rotate so this core's query half sits at columns 0..NQ-1;
            # key order is irrelevant (attention reduces over all keys)
            xr = np.concatenate([xfull[b][:, NQ:], xfull[b][:, :NQ]], axis=1)
        m["xf0"] = bf16(xr[0:128])
        m["xf1"] = bf16(xr[128:256])
        xtq = xr[:, :NQ].T + gbv_row                              # [NQ, C]
        m["xts"] = bf16(xtq.reshape(16, 128, C).transpose(1, 0, 2)
                        .reshape(128, 16 * C))
        in_maps.append(m)

    res = run_bass_kernel_spmd(_get_nc(), in_maps, list(range(NCORES)),
                               trace=_trace)
    full = np.empty((B, C, N), np.float32)
    for core in range(NCORES):
        b, h = core // 2, core % 2
        o = np.asarray(res.results[core]["outS"], dtype=np.float32)
        # outS[p, cp*1024 + j*256 + c] = out^T[cp*512 + j*128 + p, c]
        oT = o.reshape(128, NCP * 4, C).transpose(1, 0, 2).reshape(NQ, C)
        full[b][:, h * NQ:(h + 1) * NQ] = oT.T
    out = full.reshape(B, C, HH, WW)
    if _trace:
        return out, res
    return out
